# revision 27
# baseline (speedup 1.0000x reference)
"""Trainium2 Bass kernel for nn_LocalNeighborhood (retrieval_knn).

B=8 batches sharded one-per-core across 8 NeuronCores. Each core:
  - computes exact fp32 pairwise squared distances for its [4096, 4096]
    block in 32 row-tiles of [128 queries x 4096 points] (ScalarE Square
    activation with per-partition bias + DVE exact negated accumulation,
    bit-identical ordering to the jax reference),
  - extracts the 16 smallest per row with DVE max8/max_index/match_replace
    (tie behaviour identical to stable argsort),
  - gathers neighbor attr+coords with per-k indirect (descriptor) DMAs from
    a combined [attr|xyz] DRAM table, projects deltas onto the local frame
    axes on ScalarE/DVE, and writes the three outputs.

The per-tile work is software-pipelined: gather + projection of tile t-1
are interleaved behind tile t's distance/selection so the in-order Pool
sequencer never stalls on DVE selection results.
"""

import sys

sys.path.insert(0, "/opt/trn_rl_repo")

import numpy as np

B, N, K, C = 8, 4096, 16, 128
PT = 128           # queries per row-tile
NT = N // PT       # 32 row-tiles
TW = 131           # combined table row: 128 attr + 3 xyz

_CACHE = {}


def build_nc(ntiles=NT, use_gather=True, variant=None):
    variant = set((variant or '').split(','))
    import concourse.bass as bass
    import concourse.bacc as bacc
    import concourse.mybir as mybir
    from concourse.tile import TileContext

    f32 = mybir.dt.float32
    u16 = mybir.dt.uint16
    u32 = mybir.dt.uint32
    SQ = mybir.ActivationFunctionType.Square
    IDENT = mybir.ActivationFunctionType.Identity
    MUL = mybir.AluOpType.mult
    ADD = mybir.AluOpType.add
    SUB = mybir.AluOpType.subtract
    ts = bass.ts

    nc = bacc.Bacc()
    frame = nc.declare_dram_parameter("frame_b", [N, 4, 3], f32, isOutput=False)
    attr = nc.declare_dram_parameter("attr_b", [N, C], f32, isOutput=False)
    out_euc = nc.declare_dram_parameter("euclidian", [N, K, 3], f32, isOutput=True)
    out_dist = nc.declare_dram_parameter("dist", [N, K], f32, isOutput=True)
    out_attr = nc.declare_dram_parameter("nei_attr", [N, K, C], f32, isOutput=True)

    table = nc.dram_tensor("table", [N, TW], f32)
    pts_soa = nc.dram_tensor("pts_soa", [3, N], f32)

    with TileContext(nc) as tc:
        with (
            tc.tile_pool(name="big", bufs=7) as bigp,
            tc.tile_pool(name="cst", bufs=1) as cstp,
            tc.tile_pool(name="sml", bufs=4) as smlp,
            tc.tile_pool(name="gth", bufs=2) as gthp,
            tc.tile_pool(name="stg", bufs=2) as stgp,
        ):
            # ---- one-time setup: combined gather table + SoA coords ----
            for t in range(N // PT):
                stg_a = stgp.tile([PT, C], f32, tag="stg_a")
                nc.sync.dma_start(out=stg_a[:], in_=attr[ts(t, PT), :])
                nc.sync.dma_start(out=table[ts(t, PT), 0:C], in_=stg_a[:])
                stg_p = stgp.tile([PT, 3], f32, tag="stg_p")
                nc.sync.dma_start(out=stg_p[:], in_=frame[ts(t, PT), 0, :])
                nc.sync.dma_start(out=table[ts(t, PT), C : C + 3], in_=stg_p[:])
                for d in range(3):
                    nc.sync.dma_start(
                        out=pts_soa[d, ts(t, PT)], in_=stg_p[:, d : d + 1]
                    )

            # broadcast point coords across all 128 partitions: [128, 3, N]
            bcast = cstp.tile([PT, 3, N], f32, tag="bcast")
            for d in range(3):
                nc.sync.dma_start(
                    out=bcast[:, d, :], in_=pts_soa[d, :].partition_broadcast(PT)
                )

            zero = cstp.tile([PT, N], f32, tag="zero")
            nc.gpsimd.memset(zero[:], 0.0)

            # Software pipeline: gather + projection for tile t-1 are emitted
            # after tile t's distance+selection work, so the in-order Pool
            # sequencer never stalls on DVE selection before starting the
            # next tile's adds.
            prev = None
            for t in range(ntiles + 1):
                if t < ntiles:
                    # per-partition query scalars
                    ctr = smlp.tile([PT, 3], f32, tag="ctr")
                    nc.sync.dma_start(out=ctr[:], in_=frame[ts(t, PT), 0, :])
                    axes = smlp.tile([PT, 3, 3], f32, tag="axes")
                    nc.sync.dma_start(out=axes[:], in_=frame[ts(t, PT), 1:4, :])
                    negc = smlp.tile([PT, 3], f32, tag="negc")
                    nc.scalar.activation(
                        negc[:], ctr[:], mybir.ActivationFunctionType.Copy,
                        scale=-1.0,
                    )

                    # exact squared differences per coordinate (ScalarE)
                    s0 = bigp.tile([PT, N], f32, tag="big")
                    nc.scalar.activation(s0[:], bcast[:, 0, :], SQ, bias=negc[:, 0:1])
                    s1 = bigp.tile([PT, N], f32, tag="big")
                    nc.scalar.activation(s1[:], bcast[:, 1, :], SQ, bias=negc[:, 1:2])
                    s2 = bigp.tile([PT, N], f32, tag="big")
                    nc.scalar.activation(s2[:], bcast[:, 2, :], SQ, bias=negc[:, 2:3])

                    # D = (s0 + s1) + s2 on gpsimd, in place into s0;
                    # negD = 0 - D (exact negation), also in place
                    add_e = nc.vector if 'dveadd' in variant else nc.gpsimd
                    add_e.tensor_tensor(s0[:], s0[:], s1[:], op=ADD)
                    add_e.tensor_tensor(s0[:], s0[:], s2[:], op=ADD)
                    add_e.tensor_tensor(s0[:], zero[:], s0[:], op=SUB)
                    negd = s0

                    if 'nosel' in variant:
                        continue
                    # top-16 smallest distances = top-16 largest of negD
                    m1 = smlp.tile([PT, 8], f32, tag="m1")
                    nc.vector.max(m1[:], negd[:])
                    nbr = smlp.tile([PT, K], u16, tag="nbr")
                    nc.vector.max_index(nbr[:, 0:8], m1[:], negd[:])
                    nc.vector.match_replace(negd[:], m1[:], negd[:], -3.0e38)
                    m2 = smlp.tile([PT, 8], f32, tag="m2")
                    nc.vector.max(m2[:], negd[:])
                    nc.vector.max_index(nbr[:, 8:16], m2[:], negd[:])

                    # dist output = -(max values), ascending
                    dist16 = smlp.tile([PT, K], f32, tag="dist16")
                    CP = mybir.ActivationFunctionType.Copy
                    nc.scalar.activation(dist16[:, 0:8], m1[:], CP, scale=-1.0)
                    nc.scalar.activation(dist16[:, 8:16], m2[:], CP, scale=-1.0)
                    nc.sync.dma_start(out=out_dist[ts(t, PT), :], in_=dist16[:])

                    if not use_gather:
                        continue
                    cur = (t, nbr, ctr, negc, axes)
                else:
                    cur = None

                if prev is not None:
                    tp, nbrp, ctrp, negcp, axesp = prev
                    nbr32p = smlp.tile([PT, K], u32, tag="nbr32")
                    nc.gpsimd.tensor_copy(nbr32p[:], nbrp[:])
                    # gather neighbor rows: gth[p, k, :] = table[nbr[p, k], :]
                    gth = gthp.tile([PT, K, TW], f32, tag="gth")
                    for k in range(K):
                        nc.gpsimd.indirect_dma_start(
                            out=gth[:, k, :],
                            out_offset=None,
                            in_=table[:],
                            in_offset=bass.IndirectOffsetOnAxis(
                                ap=nbr32p[:, k : k + 1], axis=0
                            ),
                        )

                    # attr slice straight to output
                    nc.sync.dma_start(
                        out=out_attr[ts(tp, PT), :, :], in_=gth[:, :, 0:C]
                    )

                    # delta + frame projection
                    delta = smlp.tile([PT, 3, K], f32, tag="delta")
                    for d in range(3):
                        nc.scalar.activation(
                            delta[:, d, :], gth[:, :, C + d], IDENT,
                            bias=negcp[:, d : d + 1],
                        )
                    euc = smlp.tile([PT, K, 3], f32, tag="euc")
                    for a in range(3):
                        nc.vector.tensor_scalar_mul(
                            euc[:, :, a], delta[:, 0, :], axesp[:, a : a + 1, 0]
                        )
                        for d in (1, 2):
                            nc.vector.scalar_tensor_tensor(
                                euc[:, :, a],
                                delta[:, d, :],
                                axesp[:, a : a + 1, d],
                                euc[:, :, a],
                                op0=MUL,
                                op1=ADD,
                            )
                    nc.sync.dma_start(out=out_euc[ts(tp, PT), :, :], in_=euc[:])

                prev = cur

    nc.compile()
    return nc


def run_raw(frame: np.ndarray, attr: np.ndarray, trace: bool = False):
    from concourse import bass_utils

    key = ("nc", NT)
    if key not in _CACHE:
        _CACHE[key] = build_nc()
    nc = _CACHE[key]

    frame = np.ascontiguousarray(frame, dtype=np.float32)
    attr = np.ascontiguousarray(attr, dtype=np.float32)
    in_maps = [{"frame_b": frame[b], "attr_b": attr[b]} for b in range(B)]
    return bass_utils.run_bass_kernel_spmd(
        nc, in_maps, list(range(B)), trace=trace
    )


def kernel(frame: np.ndarray, attr: np.ndarray):
    res = run_raw(frame, attr)
    results = res.results

    euclidian = np.stack([results[b]["euclidian"] for b in range(B)])
    dist = np.stack([results[b]["dist"] for b in range(B)])[..., None]
    nei_attr = np.stack([results[b]["nei_attr"] for b in range(B)])
    return euclidian, dist, nei_attr
